# revision 111
# baseline (speedup 1.0000x reference)
"""Trainium2 Bass kernel: pre-LN multi-head attention block (B=8, L=1024,
D=1024, H=16, dk=dv=64), data-parallel over batch across 8 NeuronCores.

Per core (one batch element):
  qn   = (q - mu) * rstd  (gamma/beta folded into Wq / a per-feature bias)
  K/Q/V projections bf16 x bf16 (weights bf16); K-proj writes straight
         into two zero-padded feature-major layouts (head-a rows /
         head-b rows) so S matmuls contract over the full 128
  S^T  = K_h Q_h^T per head pair, PSUM tiles span 2 banks so exp runs
         2-wide on ACT ([128,1024] per ACTIVATE)
  P^T  = exp(S^T) bf16 (no max-subtraction: |S| <~ 7)
  O^T  = V_aug^T P^T (ones column -> sumexp row for free), normalized via
         approx-recip on DVE + GPSIMD partition_broadcast
  out  = LN(O @ Wo + q) computed token-major (stationary = O^T slices),
         no output transposes; LN + store pipeline behind the PE stream,
         elementwise spread across ACT/DVE/GPSIMD by measured cost.
  V arrives feature-major via DMA-XBAR transposes from host-cast bf16;
  K via PE transposes (XBAR too slow for the pipeline-gating operand).
"""

import numpy as np
import ml_dtypes

import concourse.bass as bass
import concourse.mybir as mybir
import concourse.tile as tile
from concourse import bacc
from concourse.dve_ops import RECIP_APPROX_FAST_CONSTS, RECIPROCAL_APPROX_FAST

P = 128
L = 1024          # tokens per batch element
D = 1024          # model dim
H = 16            # heads
HD = 64           # head dim
E = HD + 1        # head dim + sumexp column
NC = D // P       # 8 feature chunks
NT = L // P       # 8 token chunks
NQ = 2            # halves of the q/free dimension
QH = 512
EPS = 1e-6

FP32 = mybir.dt.float32
BF16 = mybir.dt.bfloat16
FP32R = mybir.dt.float32r
AX = mybir.AxisListType.X
OP = mybir.AluOpType
AF = mybir.ActivationFunctionType


def _ln_stats(nc, pool, x, eps_t):
    """Per-token mean/var of a [P, D] tile via DVE bn_stats.

    Returns (rstd, neg_mu_rstd) as [P, 1] columns for the ACT apply pass.
    All ops on DVE: no cross-engine hops until the apply.
    """
    st6 = pool.tile([P, 2, 6], FP32, tag="lnst6", bufs=3, name="st6")
    st2 = pool.tile([P, 2], FP32, tag="lnst2", bufs=3, name="st2")
    sd = pool.tile([P, 2], FP32, tag="lnsd", bufs=3, name="sd")
    x3 = x.rearrange("p (a b) -> p a b", b=QH)
    nc.vector.bn_stats(st6[:, 0, :], x3[:, 0, :])
    nc.vector.bn_stats(st6[:, 1, :], x3[:, 1, :])
    nc.vector.bn_aggr(st2, st6)                       # [mean, var]
    nc.scalar.activation(sd[:, 0:1], st2[:, 1:2], AF.Sqrt, bias=eps_t)
    nc.vector.reciprocal(sd[:, 1:2], sd[:, 0:1])      # rstd
    # -mu * rstd in one tensor_scalar: (mu * rstd) * -1
    nm = pool.tile([P, 1], FP32, tag="lnnm", bufs=3, name="nm")
    nc.vector.tensor_scalar(nm, st2[:, 0:1], sd[:, 1:2], -1.0,
                            OP.mult, OP.mult)
    return sd[:, 1:2], nm


def build_bass():
    nc = bacc.Bacc("TRN2", target_bir_lowering=False, debug=False)

    q_d = nc.dram_tensor("q", [L, D], FP32, kind="ExternalInput")
    qb_d = nc.dram_tensor("qbh", [L, D], BF16, kind="ExternalInput")
    k_d = nc.dram_tensor("kb", [L, D], BF16, kind="ExternalInput")
    v_d = nc.dram_tensor("vb", [L, D], BF16, kind="ExternalInput")
    wq_d = nc.dram_tensor("wq", [D, D], BF16, kind="ExternalInput")
    wk_d = nc.dram_tensor("wk", [D, D], BF16, kind="ExternalInput")
    wv_d = nc.dram_tensor("wv", [D, D], BF16, kind="ExternalInput")
    wo_d = nc.dram_tensor("wo", [D, D], BF16, kind="ExternalInput")
    bq_d = nc.dram_tensor("bqt", [P, NC], FP32, kind="ExternalInput")
    gb_d = nc.dram_tensor("gb", [P, D], FP32, kind="ExternalInput")
    bb_d = nc.dram_tensor("bb", [P, D], FP32, kind="ExternalInput")
    idb_d = nc.dram_tensor("identb", [P, P], BF16, kind="ExternalInput")
    on_d = nc.dram_tensor("ones1", [1, HD], FP32R, kind="ExternalInput")
    ep_d = nc.dram_tensor("epsc", [P, 1], FP32, kind="ExternalInput")
    vo_d = nc.dram_tensor("vone", [P, NT * H * E], BF16, kind="ExternalInput")
    zz_d = nc.dram_tensor("zz", [HD, NC * L], BF16, kind="ExternalInput")
    out_d = nc.dram_tensor("out", [L, D], FP32, kind="ExternalOutput")

    with tile.TileContext(nc) as tc:
        with (
            tc.tile_pool(name="const", bufs=1) as cpool,
            tc.tile_pool(name="otp", bufs=1) as otp,
            tc.tile_pool(name="wop", bufs=1) as wop,
        ):
            identb = cpool.tile([P, P], BF16, name="identb")
            ones1 = cpool.tile([1, HD], FP32R, name="ones1")
            bqt = cpool.tile([P, NC], FP32, name="bqt")
            eps_t = cpool.tile([P, 1], FP32, name="eps_t")

            OT = [otp.tile([P, L], BF16, tag="ot", bufs=8, name=f"ot{j}")
                  for j in range(H // 2)]

            with tc.tile_pool(name="qkv", bufs=1) as qkv:
                QT = qkv.tile([P, NC, L], BF16, tag="QT", name="QT")
                KTza = qkv.tile([P, NC, L], BF16, tag="KTza", name="KTza")
                KTzb = qkv.tile([P, NC, L], BF16, tag="KTzb", name="KTzb")
                Vaug = qkv.tile([P, NT, H * E], BF16, tag="Vaug", name="Vaug")

                with tc.tile_pool(name="vpool", bufs=1) as vpool:
                  with (
                    tc.tile_pool(name="actT", bufs=1) as atp,
                    tc.tile_pool(name="ps1", bufs=7, space="PSUM") as ps1,
                  ):
                    # ---- k first, then wk: the K pipeline gates everything
                    # (separate DMAs: multiple queue engines run in parallel)
                    kx = [atp.tile([P, D], BF16, tag="x", bufs=8,
                                   name=f"kx{t}") for t in range(NT)]
                    nc.sync.dma_start(identb, idb_d[:])
                    for t in range(NT):
                        nc.sync.dma_start(kx[t], k_d[t * P:(t + 1) * P, :])
                    def load_w(dram, nm):
                        """8 parallel DMAs: queue engines split the load"""
                        tiles = []
                        for i in range(NC):
                            wt = atp.tile([P, D], BF16, tag="w", bufs=12,
                                          name=f"{nm}{i}")
                            nc.sync.dma_start(wt, dram[i * P:(i + 1) * P, :])
                            tiles.append(wt)
                        return tiles

                    wk_t = load_w(wk_d, "wk")
                    nc.sync.dma_start(bqt, bq_d[:])
                    nc.sync.dma_start(eps_t, ep_d[:])

                    def transpose_bf16(xt):
                        """xt: 8 token-major [P, D] bf16 tiles -> 8
                        feature-major [P, L] bf16 chunks; 8 transposes
                        fill one PSUM bank, one drain copy per chunk"""
                        chunks = [atp.tile([P, L], BF16, tag="tch", bufs=8,
                                           name=f"tch{c}") for c in range(NC)]
                        for c in range(NC):
                            pt = ps1.tile([P, 8 * P], BF16, tag="tr",
                                          bufs=3, name="ps_tr")
                            for t in range(NT):
                                nc.tensor.transpose(
                                    pt[:, t * P:(t + 1) * P],
                                    xt[t][:, c * P:(c + 1) * P], identb)
                            if c % 2 == 1:
                                nc.scalar.activation(chunks[c], pt, AF.Copy)
                            else:
                                nc.vector.tensor_copy(chunks[c], pt)
                        return chunks

                    kT = transpose_bf16(kx)

                    # wq before the qx stream so Q-proj is never starved
                    wq_t = load_w(wq_d, "wq")

                    # qx tiles rotate through the kx buffers (tag "x");
                    # all 8 DMAs issued up front -> parallel queue engines
                    qxs = [atp.tile([P, D], BF16, tag="x", bufs=8,
                                    name=f"qx{t}") for t in range(NT)]
                    for t in range(NT):
                        nc.sync.dma_start(qxs[t], qb_d[t * P:(t + 1) * P, :])
                    qn = [atp.tile([P, D], BF16, tag="qn", bufs=8,
                                   name=f"qn{t}") for t in range(NT)]
                    for t in range(NT):
                        rstd, nm = _ln_stats(nc, atp, qxs[t], eps_t)
                        # apply on GPSIMD (idle here; ACT is the wall in
                        # this window with drains + copies)
                        nc.gpsimd.tensor_scalar(qn[t], qxs[t], rstd, nm,
                                                OP.mult, OP.add)

                    # V feature-major via DMA-XBAR transposes straight from
                    # (host-cast) bf16 DRAM; fired in the DMA-quiet window,
                    # consumed by V-proj ~40us later
                    vT = [vpool.tile([P, L], BF16, tag="vch", bufs=8,
                                    name=f"vch{c}") for c in range(NC)]
                    for c in range(NC):
                        nc.sync.dma_start(vT[c], v_d[:, c * P:(c + 1) * P],
                                          transpose=True)

                    def proj(w_tiles, act_chunks, emit_out):
                        for m in range(NC):
                            for n in range(NQ):
                                ps = ps1.tile([P, QH], FP32, tag="mm",
                                              bufs=5, name="ps_pj")
                                for i in range(NC):
                                    nc.tensor.matmul(
                                        ps,
                                        w_tiles[i][:, m * P:(m + 1) * P],
                                        act_chunks[i][:, n * QH:(n + 1) * QH],
                                        start=(i == 0), stop=(i == NC - 1),
                                    )
                                emit_out(m, n, ps)

                    def k_out(m, n, ps):
                        # split the drain across DVE and ACT
                        sl = slice(n * QH, (n + 1) * QH)
                        nc.vector.tensor_copy(KTza[0:HD, m, sl], ps[0:HD, :])
                        nc.scalar.activation(KTzb[HD:P, m, sl], ps[HD:P, :],
                                             AF.Copy)

                    proj(wk_t, kT, k_out)

                    # ---- Q transposes (bf16 input: 8 per PSUM bank) + proj

                    qnT = [atp.tile([P, L], BF16, tag="tch", bufs=8,
                                    name=f"qch{c}") for c in range(NC)]
                    for c in range(NC):
                        pt = ps1.tile([P, 8 * P], BF16, tag="tr", bufs=3,
                                      name="ps_trb")
                        for t in range(NT):
                            nc.tensor.transpose(
                                pt[:, t * P:(t + 1) * P],
                                qn[t][:, c * P:(c + 1) * P], identb)
                        if c % 2 == 0:
                            nc.vector.tensor_copy(qnT[c], pt)
                        else:
                            nc.scalar.activation(qnT[c], pt, AF.Copy)

                    def q_out(m, n, ps):
                        # + beta@Wq bias (per-partition in feature-major);
                        # ACT-only: DVE is the wall in this window
                        dst = QT[:, m, n * QH:(n + 1) * QH]
                        nc.scalar.activation(dst, ps, AF.Identity,
                                             bias=bqt[:, m:m + 1])

                    proj(wq_t, qnT, q_out)

                    # wv + consts: V-proj itself moves into the
                    # attention region so its PE window overlaps the
                    # pre-filled exps on ACT
                    wv_t = []
                    for i in range(NC):
                        wvt = vpool.tile([P, D], BF16, tag="wv", bufs=8,
                                         name=f"wv{i}")
                        nc.sync.dma_start(wvt, wv_d[i * P:(i + 1) * P, :])
                        wv_t.append(wvt)
                    nc.sync.dma_start(KTza[HD:P, :, :], zz_d[:])
                    nc.sync.dma_start(KTzb[0:HD, :, :], zz_d[:])
                    nc.sync.dma_start(Vaug, vo_d.rearrange(
                        "p (t he) -> p t he", t=NT))
                    nc.sync.dma_start(ones1, on_d[:])

                  # wo preload: lands during attention, tail starts instantly
                  wo_all = wop.tile([P, NC, D], BF16, tag="w2", bufs=1,
                                    name="wo_all")
                  nc.sync.dma_start(
                      wo_all, wo_d.rearrange("(i p) d -> p i d", p=P))
                  wo_t = [wo_all[:, i, :] for i in range(NC)]

                  # ------------- attention, software-pipelined -------------
                  # S-fills+exps run 3 iterations ahead of the PV consumers;
                  # the first 3 fills' exps saturate ACT while PE does V-proj
                  with (
                    tc.tile_pool(name="att", bufs=1) as att,
                    tc.tile_pool(name="ps2", bufs=1, space="PSUM") as ps2,
                  ):
                    iters = [(j, n) for j in range(H // 2)
                             for n in range(NQ)]

                    def s_fill(idx):
                        j, n = iters[idx]
                        qs = slice(n * QH, (n + 1) * QH)
                        PTa = att.tile([P, NC, QH], BF16, tag="pt",
                                       bufs=6, name="pta")
                        PTb = att.tile([P, NC, QH], BF16, tag="pt",
                                       bufs=6, name="ptb")
                        for sub, KZ, PT in ((0, KTza, PTa),
                                            (1, KTzb, PTb)):
                            for ii in range(NC // 2):
                                i0 = 2 * ii
                                psw = ps2.tile([P, 2 * QH], FP32,
                                               tag="s", bufs=2,
                                               name="ps_s")
                                for s in range(2):
                                    i = i0 + s
                                    ks = slice(i * P, (i + 1) * P)
                                    nc.tensor.matmul(
                                        psw[:, s * QH:(s + 1) * QH],
                                        KZ[:, j, ks], QT[:, j, qs],
                                        start=True, stop=True)
                                dst = PT[:, i0:i0 + 2, :]
                                dst = dst.rearrange("p a b -> p (a b)")
                                nc.scalar.activation(dst, psw, AF.Exp)
                        return PTa, PTb

                    pts = {i: s_fill(i) for i in range(3)}

                    # V-proj (PSUM shares ps2); drains on DVE only -- ACT
                    # is busy with the pre-filled exps
                    for t in range(NT):
                        for n in range(NQ):
                            ps = ps2.tile([P, QH], FP32, tag="vmm",
                                          bufs=2, name="ps_v")
                            for i in range(NC):
                                nc.tensor.matmul(
                                    ps,
                                    vT[i][:, t * P:(t + 1) * P],
                                    wv_t[i][:, n * QH:(n + 1) * QH],
                                    start=(i == 0), stop=(i == NC - 1),
                                )
                            dst = Vaug[:, t, n * 8 * E:(n + 1) * 8 * E]
                            dst = dst.rearrange("p (h e) -> p h e", e=E)
                            nc.vector.tensor_copy(
                                dst[:, :, 0:HD],
                                ps.rearrange("p (h d) -> p h d", d=HD))

                    for idx in range(H):
                        j, n = iters[idx]
                        ha, hb = 2 * j, 2 * j + 1
                        qs = slice(n * QH, (n + 1) * QH)
                        PTa, PTb = pts.pop(idx)
                        poa = ps2.tile([E, QH], FP32, tag="o", bufs=2,
                                       name="poa")
                        pob = ps2.tile([E, QH], FP32, tag="o", bufs=2,
                                       name="pob")
                        for i in range(NC):
                            nc.tensor.matmul(
                                poa, Vaug[:, i, ha * E:(ha + 1) * E],
                                PTa[:, i, :],
                                start=(i == 0), stop=(i == NC - 1))
                            nc.tensor.matmul(
                                pob, Vaug[:, i, hb * E:(hb + 1) * E],
                                PTb[:, i, :],
                                start=(i == 0), stop=(i == NC - 1))
                        if idx + 3 < H:
                            pts[idx + 3] = s_fill(idx + 3)
                        for sub, po in ((0, poa), (1, pob)):
                                o_tmp = att.tile([E, QH], FP32, tag="otmp",
                                                 bufs=2, name="o_tmp")
                                nc.vector.tensor_copy(o_tmp, po)
                                rin = att.tile([1, QH], FP32, tag="rin",
                                               bufs=2, name="rin")
                                nc.sync.dma_start(rin, o_tmp[HD:E, :])
                                rec = att.tile([1, QH], FP32, tag="rec",
                                               bufs=2, name="rec")
                                c = RECIP_APPROX_FAST_CONSTS
                                nc.vector._custom_dve(
                                    RECIPROCAL_APPROX_FAST, out=rec, in0=rin,
                                    s0=c["s0"], s1=c["s1"], imm2=c["imm2"])
                                # broadcast 1/sumexp across partitions on
                                # the idle GPSIMD DGE instead of a PE matmul
                                bc = att.tile([HD, QH], FP32, tag="bc",
                                              bufs=2, name="bc")
                                nc.gpsimd.partition_broadcast(bc, rec)
                                if sub == 0:
                                    nc.vector.tensor_tensor(
                                        OT[j][0:HD, qs], o_tmp[0:HD, :],
                                        bc, OP.mult)
                                else:
                                    oo = att.tile([HD, QH], BF16, tag="oo",
                                                  bufs=2, name="oo")
                                    nc.vector.tensor_tensor(
                                        oo, o_tmp[0:HD, :], bc, OP.mult)
                                    nc.sync.dma_start(OT[j][HD:P, qs], oo)

            # -------- output projection (token-major) + LN, pipelined -----
            with (
                tc.tile_pool(name="fin", bufs=1) as fin,
                tc.tile_pool(name="ps3", bufs=4, space="PSUM") as ps3,
            ):
                gamma_bc = fin.tile([P, D], FP32, name="gamma_bc")
                beta_bc = fin.tile([P, D], FP32, name="beta_bc")
                nc.sync.dma_start(gamma_bc, gb_d[:])
                nc.sync.dma_start(beta_bc, bb_d[:])
                for t in range(NT):
                    ts = slice(t * P, (t + 1) * P)
                    rt = fin.tile([P, D], FP32, tag="res", bufs=4, name="rt")
                    nc.sync.dma_start(rt, q_d[ts, :])
                    u = fin.tile([P, D], FP32, tag="oacc", bufs=3,
                                 name="oacc")
                    for n in range(NQ):
                        ps = ps3.tile([P, QH], FP32, tag="mm", bufs=4,
                                      name="ps_w")
                        for jj in range(NC):
                            nc.tensor.matmul(
                                ps,
                                OT[jj][:, ts],
                                wo_t[jj][:, n * QH:(n + 1) * QH],
                                start=(jj == 0), stop=(jj == NC - 1),
                            )
                        # fuse residual add into the PSUM drain
                        nc.vector.tensor_tensor(
                            u[:, n * QH:(n + 1) * QH], ps,
                            rt[:, n * QH:(n + 1) * QH], OP.add)
                    # final LN: stats alternate ACT-accum / DVE-bn_stats so
                    # neither engine is the trail wall; apply on ACT
                    scr = fin.tile([P, D], FP32, tag="lnscr", bufs=2,
                                   name="scr")
                    if t % 2 == 0:
                        st = fin.tile([P, 8], FP32, tag="lnst", bufs=3,
                                      name="lnst")
                        nc.scalar.activation(scr, u, AF.Copy,
                                             accum_out=st[:, 0:1])
                        nc.scalar.activation(scr, u, AF.Square,
                                             accum_out=st[:, 1:2])
                        nc.vector.tensor_scalar_mul(st[:, 2:3], st[:, 0:1],
                                                    1.0 / D)       # mu
                        nc.vector.tensor_tensor(st[:, 3:4], st[:, 2:3],
                                                st[:, 2:3], OP.mult)
                        nc.vector.tensor_scalar(st[:, 4:5], st[:, 1:2],
                                                1.0 / D, 0.0,
                                                OP.mult, OP.add)
                        nc.vector.tensor_tensor(st[:, 4:5], st[:, 4:5],
                                                st[:, 3:4], OP.subtract)
                        nc.scalar.activation(st[:, 5:6], st[:, 4:5],
                                             AF.Sqrt, bias=eps_t)
                        nc.vector.reciprocal(st[:, 6:7], st[:, 5:6])
                        nm = fin.tile([P, 1], FP32, tag="lnnm", bufs=3,
                                      name="nm2")
                        nc.vector.tensor_scalar(nm, st[:, 2:3], st[:, 6:7],
                                                -1.0, OP.mult, OP.mult)
                        rstd = st[:, 6:7]
                    else:
                        rstd, nm = _ln_stats(nc, fin, u, eps_t)
                    nc.scalar.activation(scr, u, AF.Identity,
                                         bias=nm, scale=rstd)
                    y = fin.tile([P, D], FP32, tag="y", bufs=3, name="y")
                    # gpsimd ops are ~2.6x DVE cost: give gpsimd 6 of 16
                    if t % 4 != 3:
                        nc.gpsimd.tensor_tensor(scr, scr, gamma_bc, OP.mult)
                    else:
                        nc.vector.tensor_tensor(scr, scr, gamma_bc, OP.mult)
                    nc.vector.tensor_tensor(y, scr, beta_bc, OP.add)
                    nc.sync.dma_start(out_d[ts, :], y)

    nc.compile()
    return nc


_CACHE = {}


def _get_nc():
    if "nc" not in _CACHE:
        _CACHE["nc"] = build_bass()
    return _CACHE["nc"]


def make_in_maps(q, k, v, Wq, Wk, Wv, Wo, gamma, beta):
    q = np.asarray(q, np.float32)
    k = np.asarray(k, np.float32)
    v = np.asarray(v, np.float32)
    gamma = np.asarray(gamma, np.float32)
    beta = np.asarray(beta, np.float32)
    # fold the 1/sqrt(dk) attention scale and the pre-LN gamma/beta into Wq
    wq_s = np.asarray(Wq, np.float32) * 0.125
    wq = (gamma[:, None] * wq_s).astype(ml_dtypes.bfloat16)
    bq = (beta @ wq_s).astype(np.float32)              # [D]
    bqt = np.ascontiguousarray(bq.reshape(NC, P).T)    # [P, NC]
    wk = np.asarray(Wk, np.float32).astype(ml_dtypes.bfloat16)
    wv = np.asarray(Wv, np.float32).astype(ml_dtypes.bfloat16)
    wo = np.asarray(Wo, np.float32).astype(ml_dtypes.bfloat16)
    vb = np.ascontiguousarray(v.astype(ml_dtypes.bfloat16))
    kb = np.ascontiguousarray(k.astype(ml_dtypes.bfloat16))
    qbh = np.ascontiguousarray(q.astype(ml_dtypes.bfloat16))
    gb = np.ascontiguousarray(np.tile(gamma[None, :], (P, 1)))
    bb = np.ascontiguousarray(np.tile(beta[None, :], (P, 1)))
    identb = np.eye(P, dtype=np.float32).astype(ml_dtypes.bfloat16)
    ones1 = np.ones((1, HD), np.float32)
    epsc = np.full((P, 1), EPS, np.float32)
    vone = np.ones((P, NT * H * E), ml_dtypes.bfloat16)
    zz = np.zeros((HD, NC * L), ml_dtypes.bfloat16)
    B = q.shape[0]
    return [
        {
            "q": np.ascontiguousarray(q[b]),
            "kb": kb[b], "qbh": qbh[b],
            "vb": vb[b],
            "wq": wq, "wk": wk, "wv": wv, "wo": wo, "bqt": bqt,
            "gb": gb, "bb": bb, "identb": identb,
            "ones1": ones1, "epsc": epsc, "vone": vone, "zz": zz,
        }
        for b in range(B)
    ]


def kernel(q, k, v, Wq, Wk, Wv, Wo, gamma, beta, trace=False):
    from concourse.bass_utils import run_bass_kernel_spmd

    nc = _get_nc()
    in_maps = make_in_maps(q, k, v, Wq, Wk, Wv, Wo, gamma, beta)
    res = run_bass_kernel_spmd(nc, in_maps, core_ids=list(range(len(in_maps))),
                               trace=trace)
    out = np.stack([r["out"] for r in res.results], axis=0)
    if trace:
        return out, res
    return out
